# revision 17
# baseline (speedup 1.0000x reference)
"""GNN message-passing edge scorer on 8 TRN2 NeuronCores.

Model: out[e] = relu(concat(U[src[e]], M[dst[e]]) @ W1 + b1) @ W2 + b2
  U, M: [100000, 128] f32 node tables; edge_index: [2, 1000000] int32.

Strategy (sharding_hint: edge-parallel over E, tables replicated):
  - Host marshals each core's 125k edges into 16 buckets by
    (src//25000, dst//25000) so chunk-local indices fit int16 for the
    HW dma_gather instruction; builds the wrapped [16, n/16] idx layout
    it requires (replicated over 128 partitions for the 8 Q7 cores).
  - NEFF-A (8-core SPMD): each core casts its 1/8 shard of both node
    tables f32->fp16 (DMA cast); host concatenates the shards.
  - NEFF-B (8-core SPMD): per bucket, dma_gather(transpose=True) pulls
    gathered rows FEATURE-major into SBUF straight from HBM; PE runs
    W1top/W1bot matmuls accumulating in PSUM, ACT applies relu+b1, PE
    contracts with W2, ACT adds b2 -> one f32 per edge.
  - Host applies the inverse bucket permutation to assemble the output.

HW constraints honored (found experimentally):
  - dma_gather cannot source Internal scratchpad DRAM -> fp16 tables
    travel host-side between the two NEFFs (ExternalInput in NEFF-B).
  - dma_gather(transpose=True) flips the SDMA xbar mode; any plain DMA
    in flight concurrently deadlocks the device -> all input DMAs are
    fenced (explicit deps) before the first gather and the single
    output DMA happens after the last gather's consumers.
  - single_packet=True corrupts/hangs above ~640 indices -> use False.
"""

import numpy as np

N_NODES = 100000
H = 128
N_CORES = 8
N_CHUNKS = 4
CHUNK = N_NODES // N_CHUNKS          # 25000 rows, int16-addressable
SHARD = N_NODES // N_CORES           # 12500 rows cast per core
N_BUCKETS = N_CHUNKS * N_CHUNKS
BLK = 512                            # PE moving-operand block (f32 PSUM bank)
MAX_CALL = 4096                      # indices per dma_gather call
# NOTE: striping gathers across num_swdge_queues>1 (queue_num=1..3) returns
# corrupted data on this runtime — keep everything on queue 0.

_cache = {}


def _build_neff_a():
    return _build_neff_a_reps(1)


def _build_neff_a_reps(reps):
    import concourse.bacc as bacc
    import concourse.mybir as mybir
    import concourse.tile as tile

    f32 = mybir.dt.float32
    fp16 = mybir.dt.float16
    nc = bacc.Bacc("TRN2", target_bir_lowering=False, debug=False,
                   num_devices=N_CORES)
    ush = nc.dram_tensor("ushard", [SHARD, H], f32, kind="ExternalInput")
    msh = nc.dram_tensor("mshard", [SHARD, H], f32, kind="ExternalInput")
    uo = nc.dram_tensor("u16o", [SHARD, H], fp16, kind="ExternalOutput")
    mo = nc.dram_tensor("m16o", [SHARD, H], fp16, kind="ExternalOutput")
    with tile.TileContext(nc) as tc:
        nc.gpsimd.dma_start(uo[:], ush[:])
        nc.gpsimd.dma_start(mo[:], msh[:])
        if reps > 1:
            with tc.For_i(0, reps - 1):
                nc.gpsimd.dma_start(uo[:], ush[:])
                nc.gpsimd.dma_start(mo[:], msh[:])
    nc.compile()
    return nc


def _build_neff_b(b_sizes, reps=1):
    """b_sizes: list of 16 padded bucket sizes (multiples of BLK, may be 0).

    Output layout: out [128, out_cols] f32; 512-edge block gk lands at
    [gk % 128, (gk // 128) * 512 : ...].
    """
    import concourse.bacc as bacc
    import concourse.mybir as mybir
    import concourse.tile as tile
    from concourse.tile_rust import add_dep_helper

    f32 = mybir.dt.float32
    fp16 = mybir.dt.float16
    i16 = mybir.dt.int16
    ACT = mybir.ActivationFunctionType

    tot = sum(b_sizes)
    out_cols = tot // 128

    nc = bacc.Bacc("TRN2", target_bir_lowering=False, debug=False,
                   num_devices=N_CORES)
    u16 = nc.dram_tensor("u16", [N_NODES, H], fp16, kind="ExternalInput")
    m16 = nc.dram_tensor("m16", [N_NODES, H], fp16, kind="ExternalInput")
    uidx = nc.dram_tensor("uidx", [128, tot // 16], i16, kind="ExternalInput")
    midx = nc.dram_tensor("midx", [128, tot // 16], i16, kind="ExternalInput")
    wpack = nc.dram_tensor("wpack", [H, 259], f32, kind="ExternalInput")
    out = nc.dram_tensor("out", [128, out_cols], f32, kind="ExternalOutput")

    with tile.TileContext(nc) as tc:
        with (
            tc.tile_pool(name="g", bufs=3) as gpool,
            tc.tile_pool(name="w", bufs=1) as wpool,
            tc.tile_pool(name="h", bufs=4) as hpool,
            tc.tile_pool(name="o", bufs=1) as opool,
            tc.tile_pool(name="ps", bufs=4, space="PSUM") as pp,
            tc.tile_pool(name="ps1", bufs=4, space="PSUM") as pp1,
            tc.tile_pool(name="ix", bufs=1) as idxp,
        ):
            wsb = wpool.tile([H, 259], f32, tag="wsb")
            dma_w = nc.sync.dma_start(wsb[:], wpack[:])
            uix = idxp.tile([128, tot // 16], i16, tag="uix")
            mix = idxp.tile([128, tot // 16], i16, tag="mix")
            dma_ui = nc.sync.dma_start(uix[:], uidx[:])
            dma_mi = nc.sync.dma_start(mix[:], midx[:])

            wq = wpool.tile([H, 257], fp16, tag="wq")
            cast_act = nc.scalar.activation(wq[:], wsb[:, 0:257], ACT.Copy)
            b1_t = wsb[:, 257:258]
            b2_t = wsb[:, 258:259]
            dummy = wpool.tile([1, 1], f32, tag="dummy")
            d1 = nc.scalar.activation(dummy[:], wsb[0:1, 0:1], ACT.Relu,
                                      bias=wsb[0:1, 258:259], scale=1.0)
            d2 = nc.scalar.activation(dummy[:], wsb[0:1, 0:1], ACT.Identity,
                                      bias=wsb[0:1, 258:259], scale=1.0)

            o_sb = opool.tile([128, out_cols], f32, tag="osb")

            def body(fence=True):
                first = [fence]

                def emit_gather(tile_ap, table, idx_sb, col0, n):
                    g = nc.gpsimd.dma_gather(
                        tile_ap, table, idx_sb[:, col0 // 16:(col0 + n) // 16],
                        num_idxs=n, num_idxs_reg=n, elem_size=H,
                        transpose=True, single_packet=False)
                    if first[0]:
                        first[0] = False
                        for dep in (cast_act, d1, d2, dma_w, dma_ui, dma_mi):
                            add_dep_helper(g.ins, dep.ins, sync=True,
                                           reason="fence plain DMA vs xbar gather")
                    return g


                off = 0
                gk = 0
                for ab in range(N_BUCKETS):
                    bsz = b_sizes[ab]
                    if bsz == 0:
                        continue
                    a, b = divmod(ab, N_CHUNKS)
                    usrc = u16[a * CHUNK:(a + 1) * CHUNK, :]
                    msrc = m16[b * CHUNK:(b + 1) * CHUNK, :]
                    for c0 in range(0, bsz, MAX_CALL):
                        n = min(MAX_CALL, bsz - c0)
                        ug = gpool.tile([128, 1, MAX_CALL], fp16, tag="ug")
                        mg = gpool.tile([128, 1, MAX_CALL], fp16, tag="mg")
                        emit_gather(ug[:, :, :n], usrc, uix, off + c0, n)
                        emit_gather(mg[:, :, :n], msrc, mix, off + c0, n)
                        for k in range(n // BLK):
                            sl = slice(k * BLK, (k + 1) * BLK)
                            ps = pp.tile([128, BLK], f32, tag="ps")
                            nc.tensor.matmul(ps[:], wq[:, 0:128], ug[:, 0, sl],
                                             start=True, stop=False)
                            nc.tensor.matmul(ps[:], wq[:, 128:256], mg[:, 0, sl],
                                             start=False, stop=True)
                            h_t = hpool.tile([128, BLK], fp16, tag="h")
                            nc.scalar.activation(h_t[:], ps[:], ACT.Relu,
                                                 bias=b1_t, scale=1.0)
                            # W2 contraction: H slices as stationary operand so
                            # 128 edges land on partitions; psum2[:, j] holds
                            # edges k*512 + j*128 .. +127.
                            ops = pp1.tile([128, 4], f32, tag="ops")
                            for j in range(4):
                                nc.tensor.matmul(
                                    ops[:, j:j + 1],
                                    h_t[:, j * 128:(j + 1) * 128],
                                    wq[:, 256:257], start=True, stop=True)
                            nc.scalar.activation(o_sb[:, gk * 4:(gk + 1) * 4],
                                                 ops[:], ACT.Identity,
                                                 bias=b2_t, scale=1.0)
                            gk += 1
                    off += bsz

            if reps == 1:
                body()
            else:
                body()  # first iteration carries the DMA fence
                with tc.For_i(0, reps - 1):
                    body(fence=False)
            nc.sync.dma_start(out[:], o_sb[:])
    nc.compile()
    return nc, out_cols


def _marshal(edge_index):
    """Bucket each core's edges; returns per-core device idx arrays and
    the info needed to invert the permutation on the host."""
    E = edge_index.shape[1]
    esh = E // N_CORES
    per_core = []
    counts = np.zeros((N_CORES, N_BUCKETS), dtype=np.int64)
    for c in range(N_CORES):
        src = np.asarray(edge_index[0, c * esh:(c + 1) * esh])
        dst = np.asarray(edge_index[1, c * esh:(c + 1) * esh])
        key = (src // CHUNK) * N_CHUNKS + (dst // CHUNK)
        order = np.argsort(key, kind="stable")
        sk = key[order]
        counts[c] = np.bincount(sk, minlength=N_BUCKETS)
        per_core.append((src, dst, order, sk))
    b_sizes = [int(-(-int(counts[:, ab].max()) // BLK) * BLK)
               if counts[:, ab].max() > 0 else 0 for ab in range(N_BUCKETS)]
    tot = sum(b_sizes)

    cores = []
    for c in range(N_CORES):
        src, dst, order, sk = per_core[c]
        ulocal = np.zeros(tot, dtype=np.int16)
        mlocal = np.zeros(tot, dtype=np.int16)
        # host position of padded-stream slot -> original edge index (or -1)
        inv = np.full(tot, -1, dtype=np.int64)
        off = 0
        pos = 0
        for ab in range(N_BUCKETS):
            bsz = b_sizes[ab]
            if bsz == 0:
                continue
            cnt = int(counts[c, ab])
            a, b = divmod(ab, N_CHUNKS)
            sel = order[pos:pos + cnt]
            pos += cnt
            ulocal[off:off + cnt] = (src[sel] - a * CHUNK).astype(np.int16)
            mlocal[off:off + cnt] = (dst[sel] - b * CHUNK).astype(np.int16)
            # padded slots keep idx 0 (valid row of the chunk, result unused)
            inv[off:off + cnt] = sel
            off += bsz

        def wrap(arr):
            w = arr.reshape(tot // 16, 16).T
            return np.ascontiguousarray(np.tile(w, (8, 1)))

        cores.append({"uidx": wrap(ulocal), "midx": wrap(mlocal), "inv": inv})
    return b_sizes, tot, cores


def kernel(user_features, movie_features, edge_index, W1, b1, W2, b2):
    from concourse.bass_utils import run_bass_kernel_spmd

    user_features = np.ascontiguousarray(user_features, dtype=np.float32)
    movie_features = np.ascontiguousarray(movie_features, dtype=np.float32)
    ei = np.ascontiguousarray(edge_index)
    E = ei.shape[1]
    esh = E // N_CORES

    # ---- NEFF-A: device-side f32 -> fp16 cast of the node tables ----
    if "A" not in _cache:
        _cache["A"] = _build_neff_a()
    nca = _cache["A"]
    in_a = [{"ushard": user_features[c * SHARD:(c + 1) * SHARD],
             "mshard": movie_features[c * SHARD:(c + 1) * SHARD]}
            for c in range(N_CORES)]
    res_a = run_bass_kernel_spmd(nca, in_a, core_ids=list(range(N_CORES)))
    U16 = np.concatenate([res_a.results[c]["u16o"] for c in range(N_CORES)])
    M16 = np.concatenate([res_a.results[c]["m16o"] for c in range(N_CORES)])

    # ---- host marshalling of edges ----
    b_sizes, tot, cores = _marshal(ei)

    key_b = ("B", tuple(b_sizes))
    if key_b not in _cache:
        _cache[key_b] = _build_neff_b(b_sizes)
    ncb, out_cols = _cache[key_b]

    WP = np.zeros((H, 259), dtype=np.float32)
    WP[:, 0:128] = np.asarray(W1, dtype=np.float32)[:H]
    WP[:, 128:256] = np.asarray(W1, dtype=np.float32)[H:]
    WP[:, 256] = np.asarray(W2, dtype=np.float32)[:, 0]
    WP[:, 257] = np.asarray(b1, dtype=np.float32)
    WP[:, 258] = np.asarray(b2, dtype=np.float32)[0]

    in_b = [{"u16": U16, "m16": M16, "uidx": cores[c]["uidx"],
             "midx": cores[c]["midx"], "wpack": WP} for c in range(N_CORES)]
    res_b = run_bass_kernel_spmd(ncb, in_b, core_ids=list(range(N_CORES)))

    # ---- host inverse permutation ----
    # padded-stream slot s lives at device out[s % 128, s // 128]
    out = np.empty(E, dtype=np.float32)
    s = np.arange(tot)
    flat_pos = (s % 128) * out_cols + s // 128
    for c in range(N_CORES):
        vals = res_b.results[c]["out"].reshape(-1)[flat_pos]
        inv = cores[c]["inv"]
        mask = inv >= 0
        out[c * esh + inv[mask]] = vals[mask]
    return out
